# revision 4
# baseline (speedup 1.0000x reference)
"""BaseLSSFPN voxel pooling (LSS lift-splat scatter-add) on 8 Trainium2 cores.

Gather-free streaming design:
 - Sharding: 8 cores = (2 batches) x (4 y-quarters of the BEV grid). Each
   core consumes all 6 cameras of its batch but only the frustum points
   whose voxel-y lands in its 32-row quarter. Core outputs are disjoint
   slabs of the BEV grid -> no collective at all.
 - Host (index/layout prep): filters + sorts its quarter's points by voxel,
   then lays out three slot-ordered streams, padded per y-row to 128-slot
   tiles: context rows (fp16), shifted depth logits q = z - logsumexp(z)
   (fp16; the softmax log-denominator per frustum column is folded in, so
   the device-side exp(q) is exactly the softmax weight), and the voxel-x
   coordinate (fp16). Streams are partition-major so the device reads them
   with plain large linear DMAs - no dma_gather, no SWDGE descriptors.
 - Device per 128-slot tile: w = exp(q) on Act; DVE/Pool (alternating)
   build M[p, x] = w[p] * (x == xrel[p]) in one fused scalar_tensor_tensor;
   PE accumulates M^T @ ctx_rows into the y-row's PSUM tile (31-ish matmuls
   per BEV row); Act drains finished rows to SBUF; per-row DMAs write out.
 - Host: transposes each core's [x, y, c] slab into the output.
"""

import math

import numpy as np

import concourse.bass as bass
import concourse.bacc as bacc
import concourse.mybir as mybir
from concourse.tile import TileContext
from concourse.bass_utils import run_bass_kernel_spmd

# problem geometry
VX = VY = VZ = 128
B, NCAMS, D, H, W, C = 2, 6, 112, 16, 44, 80
NCORES = 8
NQ = 4                 # y-quarters per batch
YBLK = VY // NQ        # 32 BEV rows per core
HWB = NCAMS * H * W    # 4224 frustum columns per batch
CT = 16                # slot-tiles per input chunk (328 KB DMA)

f16 = mybir.dt.float16
f32 = mybir.dt.float32

LAST_RESULT = None


def _plan_core(k, depth_logits, context, geom_xyz):
    b, q = k // NQ, k % NQ
    dl = depth_logits[b * NCAMS:(b + 1) * NCAMS]                    # (6,D,H,W)
    zt = dl.transpose(0, 2, 3, 1).reshape(HWB, D)                   # [hw, d]
    ct = context[b * NCAMS:(b + 1) * NCAMS].transpose(0, 2, 3, 1).reshape(HWB, C)
    zmax = zt.max(axis=1)
    lse = zmax + np.log(np.exp(zt - zmax[:, None]).sum(axis=1))     # [hw]

    g = geom_xyz[b].transpose(0, 2, 3, 1, 4).reshape(HWB, D, 3).astype(np.int64)
    gx, gy, gz = g[..., 0], g[..., 1], g[..., 2]
    ok = (
        (gx >= 0) & (gx < VX) & (gy >= 0) & (gy < VY)
        & (gz >= 0) & (gz < VZ)
        & (gy >= YBLK * q) & (gy < YBLK * (q + 1))
    )
    hw_i, d_i = np.nonzero(ok)
    vox = (gy[ok] - YBLK * q) * VX + gx[ok]
    order = np.argsort(vox, kind="stable")
    vox, hw_i, d_i = vox[order], hw_i[order], d_i[order]
    counts = np.bincount(vox // VX, minlength=YBLK)
    qv = (zt[hw_i, d_i] - lse[hw_i]).astype(np.float32)
    return dict(
        b=b, q=q, ct16=ct.astype(np.float16), vox=vox, hw=hw_i, qv=qv,
        counts=counts, mt=max(1, math.ceil(counts.max() / 128)),
    )


def _fill_streams(plan, mt):
    spb = mt * 128                      # slots per y-row block
    slots = YBLK * spb
    tiles = slots // 128
    counts, vox = plan["counts"], plan["vox"]
    starts = np.zeros(YBLK, np.int64)
    starts[1:] = np.cumsum(counts)[:-1]
    rank = np.arange(len(vox)) - starts[vox // VX]
    slot = (vox // VX) * spb + rank

    qs = np.full(slots, -30.0, np.float32)   # exp() -> 0 for pad slots
    qs[slot] = plan["qv"]
    xr = np.zeros(slots, np.float32)
    xr[slot] = (vox % VX).astype(np.float32)
    ctxg = np.zeros((slots, C), np.float16)
    ctxg[slot] = plan["ct16"][plan["hw"]]

    # partition-major layouts: [128, tiles(, C)] so per-partition DMA reads
    # are contiguous
    qs = np.ascontiguousarray(qs.reshape(tiles, 128).T).astype(np.float16)
    xr = np.ascontiguousarray(xr.reshape(tiles, 128).T).astype(np.float16)
    ctxg = np.ascontiguousarray(ctxg.reshape(tiles, 128, C).transpose(1, 0, 2))
    iotap = np.tile(np.arange(128, dtype=np.float16), (128, 1))
    return dict(ctxg=ctxg, qsel=qs, xrel=xr, iotap=iotap)


def _build_nc(mt, reps=1):
    tiles = YBLK * mt
    assert tiles % CT == 0
    nc = bacc.Bacc(
        "TRN2", target_bir_lowering=False, debug=False, num_devices=NCORES,
    )
    ctxg_h = nc.dram_tensor("ctxg", [128, tiles, C], f16, kind="ExternalInput")
    qsel_h = nc.dram_tensor("qsel", [128, tiles], f16, kind="ExternalInput")
    xrel_h = nc.dram_tensor("xrel", [128, tiles], f16, kind="ExternalInput")
    iota_h = nc.dram_tensor("iotap", [128, 128], f16, kind="ExternalInput")
    bev_h = nc.dram_tensor("bev", [128, YBLK * C], f32, kind="ExternalOutput")

    with TileContext(nc) as tc:
        with (
            tc.tile_pool(name="consts", bufs=1) as cpool,
            tc.tile_pool(name="gath", bufs=6) as gpool,
            tc.tile_pool(name="m8", bufs=24) as mpool,
            tc.tile_pool(name="psum", bufs=6, space="PSUM") as psum_pool,
        ):
            qsel_t = cpool.tile([128, tiles], f16)
            xrel_t = cpool.tile([128, tiles], f16)
            iota_t = cpool.tile([128, 128], f16)
            w_t = cpool.tile([128, tiles], f16)
            bev_sb = cpool.tile([128, YBLK * C], f32)
            nc.sync.dma_start(out=qsel_t[:], in_=qsel_h[:])
            nc.sync.dma_start(out=xrel_t[:], in_=xrel_h[:])
            nc.sync.dma_start(out=iota_t[:], in_=iota_h[:])

            def body():
                nc.scalar.activation(
                    out=w_t[:], in_=qsel_t[:],
                    func=mybir.ActivationFunctionType.Exp,
                )
                ps = None
                for call in range(tiles // CT):
                    t0 = call * CT
                    gt = gpool.tile([128, CT, C], f16, tag="gt")
                    nc.sync.dma_start(out=gt[:], in_=ctxg_h[:, t0:t0 + CT, :])
                    for j in range(CT):
                        t = t0 + j
                        blk, jj = t // mt, t % mt
                        m8 = mpool.tile([128, 128], f16, tag="m8")
                        nc.vector.scalar_tensor_tensor(
                            out=m8[:], in0=iota_t[:],
                            scalar=xrel_t[:, t:t + 1],
                            in1=w_t[:, t:t + 1].to_broadcast([128, 128]),
                            op0=mybir.AluOpType.is_equal,
                            op1=mybir.AluOpType.mult,
                        )
                        if jj == 0:
                            ps = psum_pool.tile([128, C], f32, tag="ps")
                        nc.tensor.matmul(
                            out=ps[:], lhsT=m8[:], rhs=gt[:, j, :],
                            start=(jj == 0), stop=(jj == mt - 1),
                        )
                        if jj == mt - 1:
                            sl = slice(blk * C, (blk + 1) * C)
                            nc.scalar.copy(out=bev_sb[:, sl], in_=ps[:])
                            nc.sync.dma_start(out=bev_h[:, sl], in_=bev_sb[:, sl])

            if reps == 1:
                body()
            else:
                with tc.For_i(0, reps, 1):
                    body()

    nc.compile()
    return nc


_NC_CACHE = {}


def kernel(depth_logits, context, geom_xyz):
    depth_logits = np.asarray(depth_logits, np.float32)
    context = np.asarray(context, np.float32)
    geom_xyz = np.asarray(geom_xyz, np.int32)

    plans = [_plan_core(k, depth_logits, context, geom_xyz) for k in range(NCORES)]
    mt = max(p["mt"] for p in plans)
    key = (mt, 1)
    if key not in _NC_CACHE:
        _NC_CACHE[key] = _build_nc(mt)
    nc = _NC_CACHE[key]

    in_maps = [_fill_streams(p, mt) for p in plans]
    res = run_bass_kernel_spmd(nc, in_maps, core_ids=list(range(NCORES)))
    global LAST_RESULT
    LAST_RESULT = res

    out = np.zeros((B, C, VY, VX), np.float32)
    for k in range(NCORES):
        part = res.results[k]["bev"].reshape(128, YBLK, C)   # [x, y, c]
        b, q = k // NQ, k % NQ
        out[b, :, YBLK * q:YBLK * (q + 1), :] = part.transpose(2, 1, 0)
    return out


# revision 5
# speedup vs baseline: 1.2746x; 1.2746x over previous
"""BaseLSSFPN voxel pooling on 8 Trainium2 cores - v65: 32-wide x-banded.

Gather-free streaming design:
 - 8 cores = (2 batches) x (4 y-quarters). Core outputs are disjoint BEV
   slabs -> no collective.
 - Host: per core, filter points to its quarter, sort by voxel, bucket into
   (BEV row y) x (32-wide x-band) segments. Segment s gets cap[s] 128-slot
   tiles where cap = max over cores (same NEFF for all 8 cores). Streams
   (ctx rows f16, q = z - lse f16, x%32 f16) are laid out partition-major
   and read with plain linear DMAs.
 - Device per block (BEV row): batched 2-pass one-hot build M[p,t,x32] =
   w * (x == xrel) alternating DVE/Pool across blocks; per segment,
   matmuls accumulate its tiles into a 32-partition PSUM slice (explicit
   tile_position - bases 0/32/64/96); Act copies the row out; per-row DMA.
 - exp(q) on Act gives the softmax weight (log-denominator folded into q
   on host, all per-point math on device).
"""

import math

import numpy as np

import concourse.bass as bass
import concourse.bacc as bacc
import concourse.mybir as mybir
from concourse.tile import TileContext
from concourse.bass_utils import run_bass_kernel_spmd

# problem geometry
VX = VY = VZ = 128
B, NCAMS, D, H, W, C = 2, 6, 112, 16, 44, 80
NCORES = 8
NQ = 4                 # y-quarters per batch
YBLK = VY // NQ        # 32 BEV rows per core
HWB = NCAMS * H * W    # 4224 frustum columns per batch
XW = 32                # x values per segment (PSUM quadrant width)
NSEG = VX // XW        # 4 segments per block

f16 = mybir.dt.float16
f32 = mybir.dt.float32

LAST_RESULT = None


def _plan_core(k, depth_logits, context, geom_xyz):
    b, q = k // NQ, k % NQ
    dl = depth_logits[b * NCAMS:(b + 1) * NCAMS]                    # (6,D,H,W)
    zt = dl.transpose(0, 2, 3, 1).reshape(HWB, D)                   # [hw, d]
    ct = context[b * NCAMS:(b + 1) * NCAMS].transpose(0, 2, 3, 1).reshape(HWB, C)
    zmax = zt.max(axis=1)
    lse = zmax + np.log(np.exp(zt - zmax[:, None]).sum(axis=1))     # [hw]

    g = geom_xyz[b].transpose(0, 2, 3, 1, 4).reshape(HWB, D, 3).astype(np.int64)
    gx, gy, gz = g[..., 0], g[..., 1], g[..., 2]
    ok = (
        (gx >= 0) & (gx < VX) & (gy >= 0) & (gy < VY)
        & (gz >= 0) & (gz < VZ)
        & (gy >= YBLK * q) & (gy < YBLK * (q + 1))
    )
    hw_i, d_i = np.nonzero(ok)
    vox = (gy[ok] - YBLK * q) * VX + gx[ok]
    order = np.argsort(vox, kind="stable")
    vox, hw_i, d_i = vox[order], hw_i[order], d_i[order]
    qv = (zt[hw_i, d_i] - lse[hw_i]).astype(np.float32)

    seg = (vox // VX) * NSEG + (vox % VX) // XW      # sorted, [0, YBLK*NSEG)
    cseg = np.bincount(seg, minlength=YBLK * NSEG)
    return dict(
        b=b, q=q, ct16=ct.astype(np.float16), vox=vox, hw=hw_i, qv=qv,
        seg=seg, cseg=cseg,
    )


def _fill_streams(plan, cap):
    capt = cap * 128                    # slots per segment
    seg_slot0 = np.zeros(len(cap) + 1, np.int64)
    seg_slot0[1:] = np.cumsum(capt)
    slots = int(seg_slot0[-1])
    tiles = slots // 128
    vox, seg = plan["vox"], plan["seg"]
    cseg = plan["cseg"]
    assert (cseg <= capt).all()
    sseg = np.zeros(len(cap), np.int64)
    sseg[1:] = np.cumsum(cseg)[:-1]
    rank = np.arange(len(vox)) - sseg[seg]
    slot = seg_slot0[seg] + rank

    qs = np.full(slots, -30.0, np.float32)   # exp() -> 0 for pad slots
    qs[slot] = plan["qv"]
    xr = np.zeros(slots, np.float32)
    xr[slot] = (vox % XW).astype(np.float32)
    ctxg = np.zeros((slots, C), np.float16)
    ctxg[slot] = plan["ct16"][plan["hw"]]

    qs = np.ascontiguousarray(qs.reshape(tiles, 128).T).astype(np.float16)
    xr = np.ascontiguousarray(xr.reshape(tiles, 128).T).astype(np.float16)
    ctxg = np.ascontiguousarray(ctxg.reshape(tiles, 128, C).transpose(1, 0, 2))
    iotap = np.tile(np.arange(128, dtype=np.float16), (128, 1))
    return dict(ctxg=ctxg, qsel=qs, xrel=xr, iotap=iotap)


def _build_nc(cap, reps=1, skip=frozenset()):
    cap = np.asarray(cap, np.int64)
    tiles = int(cap.sum())
    nc = bacc.Bacc(
        "TRN2", target_bir_lowering=False, debug=False, num_devices=NCORES,
    )
    ctxg_h = nc.dram_tensor("ctxg", [128, tiles, C], f16, kind="ExternalInput")
    qsel_h = nc.dram_tensor("qsel", [128, tiles], f16, kind="ExternalInput")
    xrel_h = nc.dram_tensor("xrel", [128, tiles], f16, kind="ExternalInput")
    iota_h = nc.dram_tensor("iotap", [128, 128], f16, kind="ExternalInput")
    bev_h = nc.dram_tensor("bev", [128, YBLK * C], f32, kind="ExternalOutput")

    blk_t0 = [0]
    for b in range(YBLK):
        blk_t0.append(blk_t0[-1] + int(cap[b * NSEG:(b + 1) * NSEG].sum()))

    with TileContext(nc) as tc:
        with (
            tc.tile_pool(name="consts", bufs=1) as cpool,
            tc.tile_pool(name="gath", bufs=4) as gpool,
            tc.tile_pool(name="m8a", bufs=4) as mpool_v,
            tc.tile_pool(name="psum", bufs=6, space="PSUM") as psum_pool,
        ):
            qsel_t = cpool.tile([128, tiles], f16)
            xrel_t = cpool.tile([128, tiles], f16)
            iota_t = cpool.tile([128, 128], f16)
            w_t = cpool.tile([128, tiles], f16)
            bev_sb = cpool.tile([128, YBLK * C], f32)
            nc.sync.dma_start(out=qsel_t[:], in_=qsel_h[:])
            nc.sync.dma_start(out=xrel_t[:], in_=xrel_h[:])
            nc.sync.dma_start(out=iota_t[:], in_=iota_h[:])
            if "pe" in skip:
                nc.vector.memset(bev_sb[:], 0.0)

            gt_fix = None
            if "dma" in skip:
                tpb0 = blk_t0[1] - blk_t0[0]
                gt_fix = cpool.tile([128, tpb0, C], f16)
                nc.sync.dma_start(out=gt_fix[:], in_=ctxg_h[:, 0:tpb0, :])

            def body():
                nc.scalar.activation(
                    out=w_t[:], in_=qsel_t[:],
                    func=mybir.ActivationFunctionType.Exp,
                )
                for b in range(YBLK):
                    t0, t1 = blk_t0[b], blk_t0[b + 1]
                    tpb = t1 - t0
                    if "dma" in skip:
                        gt = gt_fix
                        tpb = gt_fix.shape[1]
                    else:
                        gt = gpool.tile([128, tpb, C], f16, tag="gt")
                        nc.sync.dma_start(
                            out=gt[:], in_=ctxg_h[:, t0:t1, :]
                        )
                    if "pe" in skip:
                        ps = None
                    else:
                        ps = psum_pool.tile([128, C], f32, tag="ps")
                    if "dve" in skip:
                        m8 = None
                    else:
                        # batched one-hot: DVE does the compare pass, Pool
                        # (which lacks comparison ALU ops) the weight pass
                        m8 = mpool_v.tile([128, tpb, XW], f16, tag="m8")
                        nc.vector.tensor_tensor(
                            out=m8[:],
                            in0=iota_t[:, :XW].rearrange(
                                "p (o x) -> p o x", o=1
                            ).broadcast_to([128, tpb, XW]),
                            in1=xrel_t[:, t0:t1].rearrange(
                                "p (t o) -> p t o", o=1
                            ).broadcast_to([128, tpb, XW]),
                            op=mybir.AluOpType.is_equal,
                        )
                        nc.gpsimd.tensor_tensor(
                            out=m8[:], in0=m8[:],
                            in1=w_t[:, t0:t1].rearrange(
                                "p (t o) -> p t o", o=1
                            ).broadcast_to([128, tpb, XW]),
                            op=mybir.AluOpType.mult,
                        )
                    if ps is None:
                        continue
                    ti = 0
                    for s in range(NSEG):
                        x0 = XW * s
                        nmm = int(cap[b * NSEG + s])
                        for j in range(nmm):
                            nc.tensor.matmul(
                                out=ps[x0:x0 + XW, :],
                                lhsT=(iota_t[:, :XW] if m8 is None
                                      else m8[:, ti, :]),
                                rhs=gt[:, ti % tpb, :],
                                start=(j == 0), stop=(j == nmm - 1),
                                tile_position=(0, x0),
                            )
                            ti += 1
                    sl = slice(b * C, (b + 1) * C)
                    nc.scalar.copy(out=bev_sb[:, sl], in_=ps[:])
                    nc.scalar.dma_start(out=bev_h[:, sl], in_=bev_sb[:, sl])
                if "pe" in skip:
                    nc.scalar.dma_start(out=bev_h[:], in_=bev_sb[:])

            if reps == 1:
                body()
            else:
                with tc.For_i(0, reps, 1):
                    body()

    nc.compile()
    return nc


_NC_CACHE = {}


def _caps(plans):
    cmax = np.maximum.reduce([p["cseg"] for p in plans])
    return np.maximum(1, np.ceil(cmax / 128).astype(np.int64))


def kernel(depth_logits, context, geom_xyz):
    depth_logits = np.asarray(depth_logits, np.float32)
    context = np.asarray(context, np.float32)
    geom_xyz = np.asarray(geom_xyz, np.int32)

    plans = [_plan_core(k, depth_logits, context, geom_xyz) for k in range(NCORES)]
    cap = _caps(plans)
    key = (tuple(cap), 1)
    if key not in _NC_CACHE:
        _NC_CACHE[key] = _build_nc(cap)
    nc = _NC_CACHE[key]

    in_maps = [_fill_streams(p, cap) for p in plans]
    res = run_bass_kernel_spmd(nc, in_maps, core_ids=list(range(NCORES)))
    global LAST_RESULT
    LAST_RESULT = res

    out = np.zeros((B, C, VY, VX), np.float32)
    for k in range(NCORES):
        part = res.results[k]["bev"].reshape(128, YBLK, C)   # [x, y, c]
        b, q = k // NQ, k % NQ
        out[b, :, YBLK * q:YBLK * (q + 1), :] = part.transpose(2, 1, 0)
    return out
